# revision 35
# baseline (speedup 1.0000x reference)
"""MoE 2D router kernel for 8 Trainium2 NeuronCores — v9.

Strategy (pure data parallel, batch-sharded, B=16 -> 2 batches/core):
  - A-space layout: per batch [C=16, H*W=16384] viewed as [128, 2048] with
    partition p = blk*16 + c (c contiguous within 16-partition groups,
    blk = 2048-pixel block). After PE transposes the expert axis c is
    CONTIGUOUS on the free axis (runs of 16).
  - hl = x*wg + noise*softplus(x*wn): eu0 = Exp(x; scale=wnp),
    wn = Ln1p(eu0) on ACT; nwa = noise*wn on Pool; hla = (x*wgp) + nwa in
    one DVE scalar_tensor_tensor (f32-exact, matches reference rounding).
  - Only TWO f32 PE transpose sets per chunk: hlT and rwT, where
    rw = 1/wn via reciprocal_approx_fast (18-bit, far beyond the need).
    nw is never transposed: (wg - mexcl)/wn = (u + tX)*rw - noise, using
    nw/wn == noise exactly; the host supplies noise pre-transposed
    (noiseT) as an extra input, so the whole load path stays in T-space
    with no extra PE work.
  - u = hlT - m1 in f32 (exact 0 at the argmax -> mask is (u==0));
    mkB = (u==0)*-2^100 (one fused tensor_scalar); md = u + mkB;
    s2 = max(md) = m2-m1. Power-of-2 rescaling keeps mask math exact:
    tX = mkB*(s2*2^-100) = (u==0)*(m1-m2) on Pool.
  - softmax: euT = Exp(u) bf16, S = sum over c,
    G = mkB * (-(1/S)*2^-100) on Pool (exact zeros off-argmax).
  - load: q = (u + tX)*rwT - noiseT, all f32 until the bf16 store;
    load = Erf(q) on ACT. Erf ops are dep-chained after all Exp/Ln ops:
    2 activation-table loads total.
  - Outputs (G, load) are bf16 in T-layout; the host inverts the
    permutation and upcasts while unsharding.
"""
import sys

sys.path.insert(0, "/opt/trn_rl_repo")

import numpy as np

B, C, H, W = 16, 16, 128, 128
NCORES = 8
BPC = B // NCORES           # batches per core
HW = H * W                  # 16384 pixels per (batch, channel)
NBLK = 8                    # pixel blocks per batch (HW / 2048)
FB = C * HW // 128          # free size per batch in [128, FB] layout = 2048
CHW = 1024                  # chunk width
NCH = CHW // 128            # 128-col transpose groups per chunk = 8
CPB = FB // CHW             # chunks per batch = 2
NCHUNK = BPC * CPB          # chunks per core = 4
BIG = 2.0 ** 100            # power of 2: exact rescaling
BIGR = 2.0 ** -100

_CACHE = {}


def _build(erf_name="Erf"):
    import concourse.bacc as bacc
    import concourse.mybir as mybir
    from concourse.tile import TileContext, add_dep_helper

    f32 = mybir.dt.float32
    bf16 = mybir.dt.bfloat16
    AX = mybir.AxisListType
    OP = mybir.AluOpType
    AF = mybir.ActivationFunctionType

    nc = bacc.Bacc(trn_type="TRN2", target_bir_lowering=False, debug=False,
                   num_devices=NCORES, name="moe_router")

    xd = nc.dram_tensor("x", [BPC, 128, FB], f32, kind="ExternalInput")
    nd = nc.dram_tensor("noise", [BPC, 128, FB], f32, kind="ExternalInput")
    ntd = nc.dram_tensor("noise_t", [BPC, CPB, 128, CHW], f32,
                         kind="ExternalInput")
    idf_d = nc.dram_tensor("id_f", [128, 128], f32, kind="ExternalInput")
    wgp_d = nc.dram_tensor("wgp", [128, 1], f32, kind="ExternalInput")
    wnp_d = nc.dram_tensor("wnp", [128, 1], f32, kind="ExternalInput")
    gd = nc.dram_tensor("g_out", [BPC, CPB, 128, CHW], bf16,
                        kind="ExternalOutput")
    ld = nc.dram_tensor("load_out", [BPC, CPB, 128, CHW], bf16,
                        kind="ExternalOutput")

    def gkc(ap, g=NCH):
        return ap.rearrange("p (g k c) -> p g k c", g=g, k=NBLK, c=C)

    def bcast(stat, g=NCH):
        # [128, (g k)] per-pixel stat -> [128, g, k, c] stride-0 over c
        return (stat.rearrange("p (g k) -> p g k", g=g)
                .unsqueeze(-1).broadcast_to([128, g, NBLK, C]))



    NS = NCH * NBLK  # stats free size = 64

    with TileContext(nc) as tc:
        with tc.tile_pool(name="const", bufs=1) as cpool, \
             tc.tile_pool(name="io", bufs=3) as iop, \
             tc.tile_pool(name="work", bufs=2) as wp, \
             tc.tile_pool(name="small", bufs=2) as sp, \
             tc.tile_pool(name="qpool", bufs=1) as qp, \
             tc.tile_pool(name="ps_t", bufs=2, space="PSUM") as ps_t:

            consts = [None]

            def _load_consts():
                idf = cpool.tile([128, 128], f32, tag="idf")
                nc.sync.dma_start(out=idf[:, :], in_=idf_d[:, :])
                wgp = cpool.tile([128, 1], f32, tag="wgp")
                nc.sync.dma_start(out=wgp[:, :], in_=wgp_d[:, :])
                wnp = cpool.tile([128, 1], f32, tag="wnp")
                nc.sync.dma_start(out=wnp[:, :], in_=wnp_d[:, :])
                return idf, wgp, wnp

            qts = []
            expln = []   # all exp/ln-table ACT instructions

            for chunk in range(NCHUNK):
                bb, ch = divmod(chunk, CPB)
                cs = ch * CHW

                # ---- load inputs ----
                xa = iop.tile([128, CHW], f32, tag="x")
                nc.sync.dma_start(out=xa[:, :], in_=xd[bb, :, cs:cs + CHW])
                na = iop.tile([128, CHW], f32, tag="noise")
                nc.sync.dma_start(out=na[:, :], in_=nd[bb, :, cs:cs + CHW])
                nt = iop.tile([128, CHW], f32, tag="noise_t")
                nc.sync.dma_start(out=nt[:, :], in_=ntd[bb, ch, :, :])
                if consts[0] is None:
                    consts[0] = _load_consts()
                idf, wgp, wnp = consts[0]

                # ---- A-space gates ----
                eu0 = wp.tile([128, CHW], f32, tag="eu0")
                expln.append(nc.scalar.activation(eu0[:, :], xa[:, :],
                                                  AF.Exp, scale=wnp[:, :]))
                wn = wp.tile([128, CHW], f32, tag="wn")
                expln.append(nc.scalar.activation(wn[:, :], eu0[:, :],
                                                  AF.Ln, bias=1.0))
                nwa = wp.tile([128, CHW], f32, tag="nwa")
                nc.gpsimd.tensor_tensor(nwa[:, :], na[:, :], wn[:, :],
                                        op=OP.mult)
                hla = wp.tile([128, CHW], f32, tag="hla")
                nc.vector.scalar_tensor_tensor(hla[:, :], xa[:, :],
                                               wgp[:, :], nwa[:, :],
                                               op0=OP.mult, op1=OP.add)
                rw = wp.tile([128, CHW], f32, tag="rw")
                nc.vector.reciprocal_approx_fast(rw[:, :], wn[:, :])

                # ---- PE transposes to T-space (f32) ----
                hlT = ps_t.tile([128, CHW], f32, tag="hlT")
                rwT = ps_t.tile([128, CHW], f32, tag="rwT")
                for g in range(NCH):
                    s = slice(g * 128, (g + 1) * 128)
                    nc.tensor.transpose(hlT[:, s], hla[:, s], idf[:, :])
                for g in range(NCH):
                    s = slice(g * 128, (g + 1) * 128)
                    nc.tensor.transpose(rwT[:, s], rw[:, s], idf[:, :])

                # ---- T-space stats over the contiguous expert axis ----
                m1c = sp.tile([128, NS], f32, tag="m1c")
                nc.vector.tensor_reduce(m1c[:, :], gkc(hlT[:, :]),
                                        axis=AX.X, op=OP.max)
                u = wp.tile([128, CHW], f32, tag="u")
                nc.vector.tensor_tensor(gkc(u[:, :]), gkc(hlT[:, :]),
                                        bcast(m1c[:, :]), op=OP.subtract)
                mkB = wp.tile([128, CHW], f32, tag="mkB")
                nc.vector.tensor_scalar(mkB[:, :], u[:, :], 0.0, -BIG,
                                        op0=OP.is_equal, op1=OP.mult)
                md = wp.tile([128, CHW], f32, tag="md")
                nc.vector.tensor_tensor(md[:, :], u[:, :], mkB[:, :],
                                        op=OP.add)
                s2c = sp.tile([128, NS], f32, tag="s2c")
                nc.vector.tensor_reduce(s2c[:, :], gkc(md[:, :]),
                                        axis=AX.X, op=OP.max)

                # ---- softmax value: G = (u==0) * 1/sum_c exp(u) ----
                euT = wp.tile([128, CHW], bf16, tag="euT")
                expln.append(nc.scalar.activation(euT[:, :], u[:, :],
                                                  AF.Exp))
                sS = sp.tile([128, NS], f32, tag="sS")
                nc.vector.tensor_reduce(sS[:, :], gkc(euT[:, :]),
                                        axis=AX.X, op=OP.add)
                g1 = sp.tile([128, NS], f32, tag="g1")
                nc.vector.reciprocal(g1[:, :], sS[:, :])
                g1s = sp.tile([128, NS], f32, tag="g1s")
                nc.vector.tensor_scalar_mul(g1s[:, :], g1[:, :], -BIGR)
                gt = iop.tile([128, CHW], bf16, tag="g")
                nc.gpsimd.tensor_tensor(gkc(gt[:, :]), gkc(mkB[:, :]),
                                        bcast(g1s[:, :]), op=OP.mult)
                nc.sync.dma_start(out=gd[bb, ch, :, :], in_=gt[:, :])

                # ---- load: q = (u + (u==0)*(m1-m2))*rw - noise ----
                s2s = sp.tile([128, NS], f32, tag="s2s")
                nc.vector.tensor_scalar_mul(s2s[:, :], s2c[:, :], BIGR)
                tX = wp.tile([128, CHW], f32, tag="tX")
                nc.gpsimd.tensor_tensor(gkc(tX[:, :]), gkc(mkB[:, :]),
                                        bcast(s2s[:, :]), op=OP.mult)
                numer = wp.tile([128, CHW], f32, tag="numer")
                nc.vector.tensor_tensor(numer[:, :], u[:, :], tX[:, :],
                                        op=OP.add)
                q1 = wp.tile([128, CHW], f32, tag="q1")
                nc.vector.tensor_tensor(q1[:, :], numer[:, :], rwT[:, :],
                                        op=OP.mult)
                qt = qp.tile([128, CHW], f32, tag=f"q{chunk}")
                nc.vector.tensor_tensor(qt[:, :], q1[:, :], nt[:, :],
                                        op=OP.subtract)
                qts.append((bb, ch, qt))

            # ---- erf tail: single table switch, explicitly dep-chained ----
            erf_af = getattr(AF, erf_name)
            prev = None
            for bb, ch, qt in qts:
                lt = iop.tile([128, CHW], bf16, tag="load")
                i = nc.scalar.activation(lt[:, :], qt[:, :], erf_af)
                if prev is None:
                    add_dep_helper(i.ins, expln[-1].ins, sync=True,
                                   reason="erf after last exp/ln")
                else:
                    add_dep_helper(i.ins, prev.ins, sync=True,
                                   reason="erf chain")
                prev = i
                nc.sync.dma_start(out=ld[bb, ch, :, :], in_=lt[:, :])

    nc.compile()
    _fix_act_tables(nc, mybir)
    return nc


def _fix_act_tables(nc, mybir):
    """Retarget Exp/Ln/Copy activation-table loads to the combined exp+ln
    table and Erf loads to the erf-bearing table, then drop redundant
    reloads."""
    from concourse.hw_specs import get_activation_tables
    AFT = mybir.ActivationFunctionType
    tabs = list(get_activation_tables(nc.m.arch).items())
    targets = []
    for i, (_, fs) in enumerate(tabs):
        if AFT.Exp in fs and AFT.Ln in fs:
            targets.append((i, fs))
    for i, (_, fs) in enumerate(tabs):
        if AFT.Erf in fs:
            targets.append((i, fs))
    for blk in nc.m.functions[0].blocks:
        insts = blk.instructions
        loads = [(idx, inst) for idx, inst in enumerate(insts)
                 if isinstance(inst, mybir.InstLoadActFuncSet)]
        for li, (idx, load) in enumerate(loads):
            end = loads[li + 1][0] if li + 1 < len(loads) else len(insts)
            funcs = {i2.func for i2 in insts[idx + 1:end]
                     if isinstance(i2, mybir.InstActivation)}
            if not funcs:
                continue
            for tid, fs in targets:
                if funcs.issubset(fs):
                    load.act_func_set_id = tid
                    break
        cur = None
        to_remove = []
        for inst in insts:
            if isinstance(inst, mybir.InstLoadActFuncSet):
                if inst.act_func_set_id == cur and not inst.has_wait():
                    to_remove.append(inst)
                else:
                    cur = inst.act_func_set_id
            elif isinstance(inst, mybir.InstActivation):
                assert inst.func in tabs[cur][1], (inst.func, cur)
        for inst in to_remove:
            insts.remove(inst)


def make_in_maps(x, noise, wg_param, wnoise_param):
    identity = np.eye(128, dtype=np.float32)
    wgv = np.ascontiguousarray(wg_param, dtype=np.float32).reshape(C)
    wnv = np.ascontiguousarray(wnoise_param, dtype=np.float32).reshape(C)
    # per-partition scalars for p = blk*16 + c
    wgp = np.tile(wgv, NBLK).reshape(128, 1).astype(np.float32)
    wnp = np.tile(wnv, NBLK).reshape(128, 1).astype(np.float32)
    # A-layout: [B, C, HW] -> [B, NBLK, C, FB] -> [B, 128, FB]
    x4 = np.ascontiguousarray(x, dtype=np.float32).reshape(B, C, NBLK, FB)
    xa = np.ascontiguousarray(x4.transpose(0, 2, 1, 3)).reshape(B, 128, FB)
    n4 = np.ascontiguousarray(noise, dtype=np.float32).reshape(
        B, C, NBLK, FB)
    na = np.ascontiguousarray(n4.transpose(0, 2, 1, 3)).reshape(B, 128, FB)
    # T-layout noise: noiseT[b, ch, t, g*128 + blk*16 + c]
    #   = noise[b, c, blk*2048 + ch*1024 + g*128 + t]
    n6 = n4.reshape(B, C, NBLK, CPB, NCH, 128)
    ntp = np.ascontiguousarray(n6.transpose(0, 3, 5, 4, 2, 1)).reshape(
        B, CPB, 128, CHW)
    in_maps = []
    for i in range(NCORES):
        in_maps.append({"x": xa[i * BPC:(i + 1) * BPC],
                        "noise": na[i * BPC:(i + 1) * BPC],
                        "noise_t": ntp[i * BPC:(i + 1) * BPC],
                        "id_f": identity, "wgp": wgp, "wnp": wnp})
    return in_maps


def _decode_T(arr):
    """[BPC, CPB, 128, CHW] T-layout -> [BPC, C, H, W] standard layout.

    arr[bb, ch, t, g*128 + blk*16 + c] = out[bb, c, blk*2048 + ch*1024
                                             + g*128 + t]
    """
    a = np.asarray(arr, dtype=np.float32).reshape(
        BPC, CPB, 128, NCH, NBLK, C)
    a = a.transpose(0, 5, 4, 1, 3, 2)  # [bb, c, blk, ch, g, t]
    return np.ascontiguousarray(a).reshape(BPC, C, H, W)


def kernel(x, noise, wg_param, wnoise_param):
    from concourse.bass_utils import run_bass_kernel_spmd

    if "nc" not in _CACHE:
        _CACHE["nc"] = _build()
    nc = _CACHE["nc"]
    in_maps = make_in_maps(x, noise, wg_param, wnoise_param)
    res = run_bass_kernel_spmd(nc, in_maps, list(range(NCORES)))
    G = np.empty((B, C, H, W), dtype=np.float32)
    L = np.empty((B, C, H, W), dtype=np.float32)
    for i in range(NCORES):
        G[i * BPC:(i + 1) * BPC] = _decode_T(res.results[i]["g_out"])
        L[i * BPC:(i + 1) * BPC] = _decode_T(res.results[i]["load_out"])
    return G, L


# revision 37
# speedup vs baseline: 1.0681x; 1.0681x over previous
"""MoE 2D router kernel for 8 Trainium2 NeuronCores — v9.

Strategy (pure data parallel, batch-sharded, B=16 -> 2 batches/core):
  - A-space layout: per batch [C=16, H*W=16384] viewed as [128, 2048] with
    partition p = blk*16 + c (c contiguous within 16-partition groups,
    blk = 2048-pixel block). After PE transposes the expert axis c is
    CONTIGUOUS on the free axis (runs of 16).
  - hl = x*wg + noise*softplus(x*wn): eu0 = Exp(x; scale=wnp),
    wn = Ln1p(eu0) on ACT; nwa = noise*wn on Pool; hla = (x*wgp) + nwa in
    one DVE scalar_tensor_tensor (f32-exact, matches reference rounding).
  - Only TWO f32 PE transpose sets per chunk: hlT and rwT, where
    rw = 1/wn via reciprocal_approx_fast (18-bit, far beyond the need).
    nw is never transposed: (wg - mexcl)/wn = (u + tX)*rw - noise, using
    nw/wn == noise exactly; the host supplies noise pre-transposed
    (noiseT) as an extra input, so the whole load path stays in T-space
    with no extra PE work.
  - u = hlT - m1 in f32 (exact 0 at the argmax -> mask is (u==0));
    mkB = (u==0)*-2^100 (one fused tensor_scalar); md = u + mkB;
    s2 = max(md) = m2-m1. Power-of-2 rescaling keeps mask math exact:
    tX = mkB*(s2*2^-100) = (u==0)*(m1-m2) on Pool.
  - softmax: euT = Exp(u) bf16, S = sum over c,
    G = mkB * (-(1/S)*2^-100) on Pool (exact zeros off-argmax).
  - load: q = (u + tX)*rwT - noiseT, all f32 until the bf16 store;
    load = Erf(q) on ACT. Erf ops are dep-chained after all Exp/Ln ops:
    2 activation-table loads total.
  - Outputs (G, load) are bf16 in T-layout; the host inverts the
    permutation and upcasts while unsharding.
"""
import sys

sys.path.insert(0, "/opt/trn_rl_repo")

import numpy as np

B, C, H, W = 16, 16, 128, 128
NCORES = 8
BPC = B // NCORES           # batches per core
HW = H * W                  # 16384 pixels per (batch, channel)
NBLK = 8                    # pixel blocks per batch (HW / 2048)
FB = C * HW // 128          # free size per batch in [128, FB] layout = 2048
CHW = 1024                  # chunk width
NCH = CHW // 128            # 128-col transpose groups per chunk = 8
CPB = FB // CHW             # chunks per batch = 2
NCHUNK = BPC * CPB          # chunks per core = 4
BIG = 2.0 ** 100            # power of 2: exact rescaling
BIGR = 2.0 ** -100

_CACHE = {}


def _build(erf_name="Erf"):
    import concourse.bacc as bacc
    import concourse.mybir as mybir
    from concourse.tile import TileContext, add_dep_helper

    f32 = mybir.dt.float32
    bf16 = mybir.dt.bfloat16
    AX = mybir.AxisListType
    OP = mybir.AluOpType
    AF = mybir.ActivationFunctionType

    nc = bacc.Bacc(trn_type="TRN2", target_bir_lowering=False, debug=False,
                   num_devices=NCORES, name="moe_router")

    xd = nc.dram_tensor("x", [BPC, 128, FB], f32, kind="ExternalInput")
    nd = nc.dram_tensor("noise", [BPC, 128, FB], f32, kind="ExternalInput")
    ntd = nc.dram_tensor("noise_t", [BPC, CPB, 128, CHW], f32,
                         kind="ExternalInput")
    idf_d = nc.dram_tensor("id_f", [128, 128], f32, kind="ExternalInput")
    wgp_d = nc.dram_tensor("wgp", [128, 1], f32, kind="ExternalInput")
    wnp_d = nc.dram_tensor("wnp", [128, 1], f32, kind="ExternalInput")
    gd = nc.dram_tensor("g_out", [BPC, CPB, 128, CHW], bf16,
                        kind="ExternalOutput")
    ld = nc.dram_tensor("load_out", [BPC, CPB, 128, CHW], bf16,
                        kind="ExternalOutput")

    def gkc(ap, g=NCH):
        return ap.rearrange("p (g k c) -> p g k c", g=g, k=NBLK, c=C)

    def bcast(stat, g=NCH):
        # [128, (g k)] per-pixel stat -> [128, g, k, c] stride-0 over c
        return (stat.rearrange("p (g k) -> p g k", g=g)
                .unsqueeze(-1).broadcast_to([128, g, NBLK, C]))



    NS = NCH * NBLK  # stats free size = 64

    with TileContext(nc) as tc:
        with tc.tile_pool(name="const", bufs=1) as cpool, \
             tc.tile_pool(name="io", bufs=3) as iop, \
             tc.tile_pool(name="work", bufs=2) as wp, \
             tc.tile_pool(name="small", bufs=2) as sp, \
             tc.tile_pool(name="qpool", bufs=1) as qp, \
             tc.tile_pool(name="ps_t", bufs=2, space="PSUM") as ps_t:

            consts = [None]

            def _load_consts():
                idf = cpool.tile([128, 128], f32, tag="idf")
                nc.sync.dma_start(out=idf[:, :], in_=idf_d[:, :])
                wgp = cpool.tile([128, 1], f32, tag="wgp")
                nc.sync.dma_start(out=wgp[:, :], in_=wgp_d[:, :])
                wnp = cpool.tile([128, 1], f32, tag="wnp")
                nc.sync.dma_start(out=wnp[:, :], in_=wnp_d[:, :])
                return idf, wgp, wnp

            qts = []
            expln = []   # all exp/ln-table ACT instructions

            for chunk in range(NCHUNK):
                bb, ch = divmod(chunk, CPB)
                cs = ch * CHW

                # ---- load inputs (consts first: wnp gates eu0) ----
                if consts[0] is None:
                    consts[0] = _load_consts()
                idf, wgp, wnp = consts[0]
                xa = iop.tile([128, CHW], f32, tag="x")
                nc.sync.dma_start(out=xa[:, :], in_=xd[bb, :, cs:cs + CHW])
                na = iop.tile([128, CHW], f32, tag="noise")
                nc.sync.dma_start(out=na[:, :], in_=nd[bb, :, cs:cs + CHW])
                nt = iop.tile([128, CHW], f32, tag="noise_t")
                nc.sync.dma_start(out=nt[:, :], in_=ntd[bb, ch, :, :])

                # ---- A-space gates ----
                eu0 = wp.tile([128, CHW], f32, tag="eu0")
                expln.append(nc.scalar.activation(eu0[:, :], xa[:, :],
                                                  AF.Exp, scale=wnp[:, :]))
                wn = wp.tile([128, CHW], f32, tag="wn")
                expln.append(nc.scalar.activation(wn[:, :], eu0[:, :],
                                                  AF.Ln, bias=1.0))
                nwa = wp.tile([128, CHW], f32, tag="nwa")
                nc.gpsimd.tensor_tensor(nwa[:, :], na[:, :], wn[:, :],
                                        op=OP.mult)
                hla = wp.tile([128, CHW], f32, tag="hla")
                nc.vector.scalar_tensor_tensor(hla[:, :], xa[:, :],
                                               wgp[:, :], nwa[:, :],
                                               op0=OP.mult, op1=OP.add)
                rw = wp.tile([128, CHW], f32, tag="rw")
                nc.vector.reciprocal_approx_fast(rw[:, :], wn[:, :])

                # ---- PE transposes to T-space (f32) ----
                hlT = ps_t.tile([128, CHW], f32, tag="hlT")
                rwT = ps_t.tile([128, CHW], f32, tag="rwT")
                for g in range(NCH):
                    s = slice(g * 128, (g + 1) * 128)
                    nc.tensor.transpose(hlT[:, s], hla[:, s], idf[:, :])
                    nc.tensor.transpose(rwT[:, s], rw[:, s], idf[:, :])

                # ---- T-space stats over the contiguous expert axis ----
                m1c = sp.tile([128, NS], f32, tag="m1c")
                nc.vector.tensor_reduce(m1c[:, :], gkc(hlT[:, :]),
                                        axis=AX.X, op=OP.max)
                u = wp.tile([128, CHW], f32, tag="u")
                nc.vector.tensor_tensor(gkc(u[:, :]), gkc(hlT[:, :]),
                                        bcast(m1c[:, :]), op=OP.subtract)
                mkB = wp.tile([128, CHW], f32, tag="mkB")
                nc.vector.tensor_scalar(mkB[:, :], u[:, :], 0.0, -BIG,
                                        op0=OP.is_equal, op1=OP.mult)
                md = wp.tile([128, CHW], f32, tag="md")
                nc.vector.tensor_tensor(md[:, :], u[:, :], mkB[:, :],
                                        op=OP.add)
                s2c = sp.tile([128, NS], f32, tag="s2c")
                nc.vector.tensor_reduce(s2c[:, :], gkc(md[:, :]),
                                        axis=AX.X, op=OP.max)

                # ---- softmax value: G = (u==0) * 1/sum_c exp(u) ----
                euT = wp.tile([128, CHW], bf16, tag="euT")
                expln.append(nc.scalar.activation(euT[:, :], u[:, :],
                                                  AF.Exp))
                sS = sp.tile([128, NS], f32, tag="sS")
                nc.vector.tensor_reduce(sS[:, :], gkc(euT[:, :]),
                                        axis=AX.X, op=OP.add)
                g1 = sp.tile([128, NS], f32, tag="g1")
                nc.vector.reciprocal(g1[:, :], sS[:, :])
                g1s = sp.tile([128, NS], f32, tag="g1s")
                nc.vector.tensor_scalar_mul(g1s[:, :], g1[:, :], -BIGR)
                gt = iop.tile([128, CHW], bf16, tag="g")
                nc.gpsimd.tensor_tensor(gkc(gt[:, :]), gkc(mkB[:, :]),
                                        bcast(g1s[:, :]), op=OP.mult)
                nc.sync.dma_start(out=gd[bb, ch, :, :], in_=gt[:, :])

                # ---- load: q = (u + (u==0)*(m1-m2))*rw - noise ----
                s2s = sp.tile([128, NS], f32, tag="s2s")
                nc.vector.tensor_scalar_mul(s2s[:, :], s2c[:, :], BIGR)
                tX = wp.tile([128, CHW], f32, tag="tX")
                nc.gpsimd.tensor_tensor(gkc(tX[:, :]), gkc(mkB[:, :]),
                                        bcast(s2s[:, :]), op=OP.mult)
                numer = wp.tile([128, CHW], f32, tag="numer")
                nc.vector.tensor_tensor(numer[:, :], u[:, :], tX[:, :],
                                        op=OP.add)
                q1 = wp.tile([128, CHW], f32, tag="q1")
                nc.vector.tensor_tensor(q1[:, :], numer[:, :], rwT[:, :],
                                        op=OP.mult)
                qt = qp.tile([128, CHW], bf16, tag=f"q{chunk}")
                nc.vector.tensor_tensor(qt[:, :], q1[:, :], nt[:, :],
                                        op=OP.subtract)
                qts.append((bb, ch, qt))

            # ---- erf tail: single table switch, explicitly dep-chained ----
            erf_af = getattr(AF, erf_name)
            prev = None
            for bb, ch, qt in qts:
                lt = iop.tile([128, CHW], bf16, tag="load")
                i = nc.scalar.activation(lt[:, :], qt[:, :], erf_af)
                if prev is None:
                    add_dep_helper(i.ins, expln[-1].ins, sync=True,
                                   reason="erf after last exp/ln")
                else:
                    add_dep_helper(i.ins, prev.ins, sync=True,
                                   reason="erf chain")
                prev = i
                nc.sync.dma_start(out=ld[bb, ch, :, :], in_=lt[:, :])

    nc.compile()
    _fix_act_tables(nc, mybir)
    return nc


def _fix_act_tables(nc, mybir):
    """Retarget Exp/Ln/Copy activation-table loads to the combined exp+ln
    table and Erf loads to the erf-bearing table, then drop redundant
    reloads."""
    from concourse.hw_specs import get_activation_tables
    AFT = mybir.ActivationFunctionType
    tabs = list(get_activation_tables(nc.m.arch).items())
    targets = []
    for i, (_, fs) in enumerate(tabs):
        if AFT.Exp in fs and AFT.Ln in fs:
            targets.append((i, fs))
    for i, (_, fs) in enumerate(tabs):
        if AFT.Erf in fs:
            targets.append((i, fs))
    for blk in nc.m.functions[0].blocks:
        insts = blk.instructions
        loads = [(idx, inst) for idx, inst in enumerate(insts)
                 if isinstance(inst, mybir.InstLoadActFuncSet)]
        for li, (idx, load) in enumerate(loads):
            end = loads[li + 1][0] if li + 1 < len(loads) else len(insts)
            funcs = {i2.func for i2 in insts[idx + 1:end]
                     if isinstance(i2, mybir.InstActivation)}
            if not funcs:
                continue
            for tid, fs in targets:
                if funcs.issubset(fs):
                    load.act_func_set_id = tid
                    break
        cur = None
        to_remove = []
        for inst in insts:
            if isinstance(inst, mybir.InstLoadActFuncSet):
                if inst.act_func_set_id == cur and not inst.has_wait():
                    to_remove.append(inst)
                else:
                    cur = inst.act_func_set_id
            elif isinstance(inst, mybir.InstActivation):
                assert inst.func in tabs[cur][1], (inst.func, cur)
        for inst in to_remove:
            insts.remove(inst)


def make_in_maps(x, noise, wg_param, wnoise_param):
    identity = np.eye(128, dtype=np.float32)
    wgv = np.ascontiguousarray(wg_param, dtype=np.float32).reshape(C)
    wnv = np.ascontiguousarray(wnoise_param, dtype=np.float32).reshape(C)
    # per-partition scalars for p = blk*16 + c
    wgp = np.tile(wgv, NBLK).reshape(128, 1).astype(np.float32)
    wnp = np.tile(wnv, NBLK).reshape(128, 1).astype(np.float32)
    # A-layout: [B, C, HW] -> [B, NBLK, C, FB] -> [B, 128, FB]
    x4 = np.ascontiguousarray(x, dtype=np.float32).reshape(B, C, NBLK, FB)
    xa = np.ascontiguousarray(x4.transpose(0, 2, 1, 3)).reshape(B, 128, FB)
    n4 = np.ascontiguousarray(noise, dtype=np.float32).reshape(
        B, C, NBLK, FB)
    na = np.ascontiguousarray(n4.transpose(0, 2, 1, 3)).reshape(B, 128, FB)
    # T-layout noise: noiseT[b, ch, t, g*128 + blk*16 + c]
    #   = noise[b, c, blk*2048 + ch*1024 + g*128 + t]
    n6 = n4.reshape(B, C, NBLK, CPB, NCH, 128)
    ntp = np.ascontiguousarray(n6.transpose(0, 3, 5, 4, 2, 1)).reshape(
        B, CPB, 128, CHW)
    in_maps = []
    for i in range(NCORES):
        in_maps.append({"x": xa[i * BPC:(i + 1) * BPC],
                        "noise": na[i * BPC:(i + 1) * BPC],
                        "noise_t": ntp[i * BPC:(i + 1) * BPC],
                        "id_f": identity, "wgp": wgp, "wnp": wnp})
    return in_maps


def _decode_T(arr):
    """[BPC, CPB, 128, CHW] T-layout -> [BPC, C, H, W] standard layout.

    arr[bb, ch, t, g*128 + blk*16 + c] = out[bb, c, blk*2048 + ch*1024
                                             + g*128 + t]
    """
    a = np.asarray(arr, dtype=np.float32).reshape(
        BPC, CPB, 128, NCH, NBLK, C)
    a = a.transpose(0, 5, 4, 1, 3, 2)  # [bb, c, blk, ch, g, t]
    return np.ascontiguousarray(a).reshape(BPC, C, H, W)


def kernel(x, noise, wg_param, wnoise_param):
    from concourse.bass_utils import run_bass_kernel_spmd

    if "nc" not in _CACHE:
        _CACHE["nc"] = _build()
    nc = _CACHE["nc"]
    in_maps = make_in_maps(x, noise, wg_param, wnoise_param)
    res = run_bass_kernel_spmd(nc, in_maps, list(range(NCORES)))
    G = np.empty((B, C, H, W), dtype=np.float32)
    L = np.empty((B, C, H, W), dtype=np.float32)
    for i in range(NCORES):
        G[i * BPC:(i + 1) * BPC] = _decode_T(res.results[i]["g_out"])
        L[i * BPC:(i + 1) * BPC] = _decode_T(res.results[i]["load_out"])
    return G, L


# revision 38
# speedup vs baseline: 1.0802x; 1.0113x over previous
"""MoE 2D router kernel for 8 Trainium2 NeuronCores — v9.

Strategy (pure data parallel, batch-sharded, B=16 -> 2 batches/core):
  - A-space layout: per batch [C=16, H*W=16384] viewed as [128, 2048] with
    partition p = blk*16 + c (c contiguous within 16-partition groups,
    blk = 2048-pixel block). After PE transposes the expert axis c is
    CONTIGUOUS on the free axis (runs of 16).
  - hl = x*wg + noise*softplus(x*wn): eu0 = Exp(x; scale=wnp),
    wn = Ln1p(eu0) on ACT; nwa = noise*wn on Pool; hla = (x*wgp) + nwa in
    one DVE scalar_tensor_tensor (f32-exact, matches reference rounding).
  - Only TWO f32 PE transpose sets per chunk: hlT and rwT, where
    rw = 1/wn via reciprocal_approx_fast (18-bit, far beyond the need).
    nw is never transposed: (wg - mexcl)/wn = (u + tX)*rw - noise, using
    nw/wn == noise exactly; the host supplies noise pre-transposed
    (noiseT) as an extra input, so the whole load path stays in T-space
    with no extra PE work.
  - u = hlT - m1 in f32 (exact 0 at the argmax -> mask is (u==0));
    mkB = (u==0)*-2^100 (one fused tensor_scalar); md = u + mkB;
    s2 = max(md) = m2-m1. Power-of-2 rescaling keeps mask math exact:
    tX = mkB*(s2*2^-100) = (u==0)*(m1-m2) on Pool.
  - softmax: euT = Exp(u) bf16, S = sum over c,
    G = mkB * (-(1/S)*2^-100) on Pool (exact zeros off-argmax).
  - load: q = (u + tX)*rwT - noiseT, all f32 until the bf16 store;
    load = Erf(q) on ACT. Erf ops are dep-chained after all Exp/Ln ops:
    2 activation-table loads total.
  - Outputs (G, load) are bf16 in T-layout; the host inverts the
    permutation and upcasts while unsharding.
"""
import sys

sys.path.insert(0, "/opt/trn_rl_repo")

import numpy as np

B, C, H, W = 16, 16, 128, 128
NCORES = 8
BPC = B // NCORES           # batches per core
HW = H * W                  # 16384 pixels per (batch, channel)
NBLK = 8                    # pixel blocks per batch (HW / 2048)
FB = C * HW // 128          # free size per batch in [128, FB] layout = 2048
CHW = 1024                  # chunk width
NCH = CHW // 128            # 128-col transpose groups per chunk = 8
CPB = FB // CHW             # chunks per batch = 2
NCHUNK = BPC * CPB          # chunks per core = 4
BIG = 2.0 ** 100            # power of 2: exact rescaling
BIGR = 2.0 ** -100

_CACHE = {}


def _build(erf_name="Erf"):
    import concourse.bacc as bacc
    import concourse.mybir as mybir
    from concourse.tile import TileContext, add_dep_helper

    f32 = mybir.dt.float32
    bf16 = mybir.dt.bfloat16
    AX = mybir.AxisListType
    OP = mybir.AluOpType
    AF = mybir.ActivationFunctionType

    nc = bacc.Bacc(trn_type="TRN2", target_bir_lowering=False, debug=False,
                   num_devices=NCORES, name="moe_router")

    xd = nc.dram_tensor("x", [BPC, 128, FB], f32, kind="ExternalInput")
    nd = nc.dram_tensor("noise", [BPC, 128, FB], f32, kind="ExternalInput")
    ntd = nc.dram_tensor("noise_t", [BPC, CPB, 128, CHW], f32,
                         kind="ExternalInput")
    cst_d = nc.dram_tensor("cst", [128, 130], f32, kind="ExternalInput")
    gd = nc.dram_tensor("g_out", [BPC, CPB, 128, CHW], bf16,
                        kind="ExternalOutput")
    ld = nc.dram_tensor("load_out", [BPC, CPB, 128, CHW], bf16,
                        kind="ExternalOutput")

    def gkc(ap, g=NCH):
        return ap.rearrange("p (g k c) -> p g k c", g=g, k=NBLK, c=C)

    def bcast(stat, g=NCH):
        # [128, (g k)] per-pixel stat -> [128, g, k, c] stride-0 over c
        return (stat.rearrange("p (g k) -> p g k", g=g)
                .unsqueeze(-1).broadcast_to([128, g, NBLK, C]))



    NS = NCH * NBLK  # stats free size = 64

    with TileContext(nc) as tc:
        with tc.tile_pool(name="const", bufs=1) as cpool, \
             tc.tile_pool(name="io", bufs=3) as iop, \
             tc.tile_pool(name="work", bufs=2) as wp, \
             tc.tile_pool(name="small", bufs=2) as sp, \
             tc.tile_pool(name="qpool", bufs=1) as qp, \
             tc.tile_pool(name="ps_t", bufs=2, space="PSUM") as ps_t:

            consts = [None]

            def _load_consts():
                ct = cpool.tile([128, 130], f32, tag="cst")
                nc.sync.dma_start(out=ct[:, :], in_=cst_d[:, :])
                return ct[:, :128], ct[:, 128:129], ct[:, 129:130]

            qts = []
            expln = []   # all exp/ln-table ACT instructions

            for chunk in range(NCHUNK):
                bb, ch = divmod(chunk, CPB)
                cs = ch * CHW

                # ---- load inputs (consts first: wnp gates eu0) ----
                if consts[0] is None:
                    consts[0] = _load_consts()
                idf, wgp, wnp = consts[0]
                xa = iop.tile([128, CHW], f32, tag="x")
                nc.sync.dma_start(out=xa[:, :], in_=xd[bb, :, cs:cs + CHW])
                na = iop.tile([128, CHW], f32, tag="noise")
                nc.sync.dma_start(out=na[:, :], in_=nd[bb, :, cs:cs + CHW])
                nt = iop.tile([128, CHW], f32, tag="noise_t")
                nc.sync.dma_start(out=nt[:, :], in_=ntd[bb, ch, :, :])

                # ---- A-space gates ----
                eu0 = wp.tile([128, CHW], f32, tag="eu0")
                expln.append(nc.scalar.activation(eu0[:, :], xa[:, :],
                                                  AF.Exp, scale=wnp[:, :]))
                wn = wp.tile([128, CHW], f32, tag="wn")
                expln.append(nc.scalar.activation(wn[:, :], eu0[:, :],
                                                  AF.Ln, bias=1.0))
                nwa = wp.tile([128, CHW], f32, tag="nwa")
                nc.gpsimd.tensor_tensor(nwa[:, :], na[:, :], wn[:, :],
                                        op=OP.mult)
                hla = wp.tile([128, CHW], f32, tag="hla")
                nc.vector.scalar_tensor_tensor(hla[:, :], xa[:, :],
                                               wgp[:, :], nwa[:, :],
                                               op0=OP.mult, op1=OP.add)
                rw = wp.tile([128, CHW], f32, tag="rw")
                nc.vector.reciprocal_approx_fast(rw[:, :], wn[:, :])

                # ---- PE transposes to T-space (f32) ----
                hlT = ps_t.tile([128, CHW], f32, tag="hlT")
                rwT = ps_t.tile([128, CHW], f32, tag="rwT")
                for g in range(NCH):
                    s = slice(g * 128, (g + 1) * 128)
                    nc.tensor.transpose(hlT[:, s], hla[:, s], idf[:, :])
                    nc.tensor.transpose(rwT[:, s], rw[:, s], idf[:, :])

                # ---- T-space stats over the contiguous expert axis ----
                m1c = sp.tile([128, NS], f32, tag="m1c")
                nc.vector.tensor_reduce(m1c[:, :], gkc(hlT[:, :]),
                                        axis=AX.X, op=OP.max)
                u = wp.tile([128, CHW], f32, tag="u")
                nc.vector.tensor_tensor(gkc(u[:, :]), gkc(hlT[:, :]),
                                        bcast(m1c[:, :]), op=OP.subtract)
                mkB = wp.tile([128, CHW], f32, tag="mkB")
                nc.vector.tensor_scalar(mkB[:, :], u[:, :], 0.0, -BIG,
                                        op0=OP.is_equal, op1=OP.mult)
                md = wp.tile([128, CHW], f32, tag="md")
                nc.vector.tensor_tensor(md[:, :], u[:, :], mkB[:, :],
                                        op=OP.add)
                s2c = sp.tile([128, NS], f32, tag="s2c")
                nc.vector.tensor_reduce(s2c[:, :], gkc(md[:, :]),
                                        axis=AX.X, op=OP.max)

                # ---- softmax value: G = (u==0) * 1/sum_c exp(u) ----
                euT = wp.tile([128, CHW], bf16, tag="euT")
                expln.append(nc.scalar.activation(euT[:, :], u[:, :],
                                                  AF.Exp))
                sS = sp.tile([128, NS], f32, tag="sS")
                nc.vector.tensor_reduce(sS[:, :], gkc(euT[:, :]),
                                        axis=AX.X, op=OP.add)
                g1 = sp.tile([128, NS], f32, tag="g1")
                nc.vector.reciprocal(g1[:, :], sS[:, :])
                g1s = sp.tile([128, NS], f32, tag="g1s")
                nc.vector.tensor_scalar_mul(g1s[:, :], g1[:, :], -BIGR)
                gt = iop.tile([128, CHW], bf16, tag="g")
                nc.gpsimd.tensor_tensor(gkc(gt[:, :]), gkc(mkB[:, :]),
                                        bcast(g1s[:, :]), op=OP.mult)
                nc.sync.dma_start(out=gd[bb, ch, :, :], in_=gt[:, :])

                # ---- load: q = (u + (u==0)*(m1-m2))*rw - noise ----
                s2s = sp.tile([128, NS], f32, tag="s2s")
                nc.vector.tensor_scalar_mul(s2s[:, :], s2c[:, :], BIGR)
                tX = wp.tile([128, CHW], f32, tag="tX")
                nc.gpsimd.tensor_tensor(gkc(tX[:, :]), gkc(mkB[:, :]),
                                        bcast(s2s[:, :]), op=OP.mult)
                numer = wp.tile([128, CHW], f32, tag="numer")
                nc.vector.tensor_tensor(numer[:, :], u[:, :], tX[:, :],
                                        op=OP.add)
                q1 = wp.tile([128, CHW], f32, tag="q1")
                nc.vector.tensor_tensor(q1[:, :], numer[:, :], rwT[:, :],
                                        op=OP.mult)
                qt = qp.tile([128, CHW], bf16, tag=f"q{chunk}")
                nc.vector.tensor_tensor(qt[:, :], q1[:, :], nt[:, :],
                                        op=OP.subtract)
                qts.append((bb, ch, qt))

            # ---- erf tail: single table switch, explicitly dep-chained ----
            erf_af = getattr(AF, erf_name)
            prev = None
            for bb, ch, qt in qts:
                lt = iop.tile([128, CHW], bf16, tag="load")
                i = nc.scalar.activation(lt[:, :], qt[:, :], erf_af)
                if prev is None:
                    add_dep_helper(i.ins, expln[-1].ins, sync=True,
                                   reason="erf after last exp/ln")
                else:
                    add_dep_helper(i.ins, prev.ins, sync=True,
                                   reason="erf chain")
                prev = i
                nc.sync.dma_start(out=ld[bb, ch, :, :], in_=lt[:, :])

    nc.compile()
    _fix_act_tables(nc, mybir)
    return nc


def _fix_act_tables(nc, mybir):
    """Retarget Exp/Ln/Copy activation-table loads to the combined exp+ln
    table and Erf loads to the erf-bearing table, then drop redundant
    reloads."""
    from concourse.hw_specs import get_activation_tables
    AFT = mybir.ActivationFunctionType
    tabs = list(get_activation_tables(nc.m.arch).items())
    targets = []
    for i, (_, fs) in enumerate(tabs):
        if AFT.Exp in fs and AFT.Ln in fs:
            targets.append((i, fs))
    for i, (_, fs) in enumerate(tabs):
        if AFT.Erf in fs:
            targets.append((i, fs))
    for blk in nc.m.functions[0].blocks:
        insts = blk.instructions
        loads = [(idx, inst) for idx, inst in enumerate(insts)
                 if isinstance(inst, mybir.InstLoadActFuncSet)]
        for li, (idx, load) in enumerate(loads):
            end = loads[li + 1][0] if li + 1 < len(loads) else len(insts)
            funcs = {i2.func for i2 in insts[idx + 1:end]
                     if isinstance(i2, mybir.InstActivation)}
            if not funcs:
                continue
            for tid, fs in targets:
                if funcs.issubset(fs):
                    load.act_func_set_id = tid
                    break
        cur = None
        to_remove = []
        for inst in insts:
            if isinstance(inst, mybir.InstLoadActFuncSet):
                if inst.act_func_set_id == cur and not inst.has_wait():
                    to_remove.append(inst)
                else:
                    cur = inst.act_func_set_id
            elif isinstance(inst, mybir.InstActivation):
                assert inst.func in tabs[cur][1], (inst.func, cur)
        for inst in to_remove:
            insts.remove(inst)


def make_in_maps(x, noise, wg_param, wnoise_param):
    identity = np.eye(128, dtype=np.float32)
    wgv = np.ascontiguousarray(wg_param, dtype=np.float32).reshape(C)
    wnv = np.ascontiguousarray(wnoise_param, dtype=np.float32).reshape(C)
    # per-partition scalars for p = blk*16 + c
    wgp = np.tile(wgv, NBLK).reshape(128, 1).astype(np.float32)
    wnp = np.tile(wnv, NBLK).reshape(128, 1).astype(np.float32)
    # A-layout: [B, C, HW] -> [B, NBLK, C, FB] -> [B, 128, FB]
    x4 = np.ascontiguousarray(x, dtype=np.float32).reshape(B, C, NBLK, FB)
    xa = np.ascontiguousarray(x4.transpose(0, 2, 1, 3)).reshape(B, 128, FB)
    n4 = np.ascontiguousarray(noise, dtype=np.float32).reshape(
        B, C, NBLK, FB)
    na = np.ascontiguousarray(n4.transpose(0, 2, 1, 3)).reshape(B, 128, FB)
    # T-layout noise: noiseT[b, ch, t, g*128 + blk*16 + c]
    #   = noise[b, c, blk*2048 + ch*1024 + g*128 + t]
    n6 = n4.reshape(B, C, NBLK, CPB, NCH, 128)
    ntp = np.ascontiguousarray(n6.transpose(0, 3, 5, 4, 2, 1)).reshape(
        B, CPB, 128, CHW)
    cst = np.concatenate([identity, wgp, wnp], axis=1)  # [128, 130]
    in_maps = []
    for i in range(NCORES):
        in_maps.append({"x": xa[i * BPC:(i + 1) * BPC],
                        "noise": na[i * BPC:(i + 1) * BPC],
                        "noise_t": ntp[i * BPC:(i + 1) * BPC],
                        "cst": cst})
    return in_maps


def _decode_T(arr):
    """[BPC, CPB, 128, CHW] T-layout -> [BPC, C, H, W] standard layout.

    arr[bb, ch, t, g*128 + blk*16 + c] = out[bb, c, blk*2048 + ch*1024
                                             + g*128 + t]
    """
    a = np.asarray(arr, dtype=np.float32).reshape(
        BPC, CPB, 128, NCH, NBLK, C)
    a = a.transpose(0, 5, 4, 1, 3, 2)  # [bb, c, blk, ch, g, t]
    return np.ascontiguousarray(a).reshape(BPC, C, H, W)


def kernel(x, noise, wg_param, wnoise_param):
    from concourse.bass_utils import run_bass_kernel_spmd

    if "nc" not in _CACHE:
        _CACHE["nc"] = _build()
    nc = _CACHE["nc"]
    in_maps = make_in_maps(x, noise, wg_param, wnoise_param)
    res = run_bass_kernel_spmd(nc, in_maps, list(range(NCORES)))
    G = np.empty((B, C, H, W), dtype=np.float32)
    L = np.empty((B, C, H, W), dtype=np.float32)
    for i in range(NCORES):
        G[i * BPC:(i + 1) * BPC] = _decode_T(res.results[i]["g_out"])
        L[i * BPC:(i + 1) * BPC] = _decode_T(res.results[i]["load_out"])
    return G, L
